# revision 20
# baseline (speedup 1.0000x reference)
"""Trainium2 Bass kernel for GroupNorm->cross-attention block (nn_Block_70325794504976).

Data-parallel over batch: 16 batches / 8 cores = 2 batches per core.
All matmuls in bf16 (PE 1 cyc/row, FWL weight loads); accumulation and the
residual path stay fp32, so final output error is damped by the small Wp
(0.02/sqrt(c)) relative to the exact fp32 residual.

Math (per batch):
  q  = (s*Wq) @ GN(x)  + s*bq          s = 1/sqrt(sqrt(d)), folded on host
  kv = Wkv' @ GN(ctx^T) + bkv'         k-half of Wkv/bkv pre-scaled by s on host
  wT[s,t] = k_h^T q_h                  (scores, [key, query] orientation)
  p = exp(wT + madd[s]) (madd = -1e9 masked, 0 else; stable-softmax max-sub
                         skipped: scores are O(5) so fp32 exp is safe)
  U[65,t] = [v_h; ones]^T @ p          row 64 = softmax denominator Z
  a_h = U[0:64] * (1/Z)                (gpsimd partition_broadcast for 1/Z)
  out = Wp @ a + bp + x
"""

import numpy as np

NUM_HEADS = 16
C = 1024
S = 1024          # spatial 32*32
CTXD = 2048
SK = 256
D = C // NUM_HEADS          # 64
B_PER = 2                   # batches per core
NCORES = 8
EPS = 1e-5

_cache = {}


def _build_program():
    import concourse.bacc as bacc
    import concourse.tile as tile
    from concourse import mybir

    F32 = mybir.dt.float32
    BF = mybir.dt.bfloat16
    AF = mybir.ActivationFunctionType
    ALU = mybir.AluOpType
    AX = mybir.AxisListType

    nc = bacc.Bacc("TRN2", target_bir_lowering=False)

    def din(name, shape, dt=F32):
        return nc.declare_dram_parameter(name, list(shape), dt, isOutput=False)

    x_d = din("x", [B_PER, C, S])
    ctx_d = din("ctx", [B_PER, SK, CTXD])
    madd_d = din("madd", [B_PER, SK])
    wq_d = din("wqt", [C, C], BF)           # (s*Wq).T
    wkv_d = din("wkvt", [CTXD, 2 * C], BF)  # Wkv, k-half scaled, transposed
    wp_d = din("wpt", [C, C], BF)           # Wp.T
    bq_d = din("bqs", [C])
    bkv_d = din("bkvs", [2 * C])
    bp_d = din("bps", [C])
    gx_d = din("gx", [C])
    bx_d = din("bx", [C])
    gc_d = din("gc", [CTXD])
    bc_d = din("bc", [CTXD])
    sel4_d = din("sel4", [128, 4], BF)      # p -> group p//32 one-hot
    sel2_d = din("sel2", [128, 2], BF)      # p -> group p//64 one-hot
    bc4_d = din("bc4", [4, 128], BF)        # transpose of sel4
    bc2_d = din("bc2", [2, 128], BF)
    id_d = din("ident", [128, 128])
    out_d = nc.declare_dram_parameter("out", [B_PER, C, S], F32, isOutput=True)

    NXC = C // 128            # 8  x channel chunks
    NCC = CTXD // 128         # 16 ctx channel chunks
    NKV = 2 * C // 128        # 16 kv output chunks
    NSC = SK // 128           # 2  key-sequence chunks
    NH = NUM_HEADS

    with tile.TileContext(nc) as tc:
        import contextlib
        est = contextlib.ExitStack()
        with est:
            consts = est.enter_context(tc.tile_pool(name="consts", bufs=1))
            big = est.enter_context(tc.tile_pool(name="big", bufs=8))       # x fp32 (resident for residual)
            xbp = est.enter_context(tc.tile_pool(name="xbp", bufs=8))       # x bf16
            asbp = est.enter_context(tc.tile_pool(name="asbp", bufs=8))     # a chunks bf16
            ksbp = est.enter_context(tc.tile_pool(name="ksbp", bufs=16))
            vaugp = est.enter_context(tc.tile_pool(name="vaugp", bufs=2))
            wstr = est.enter_context(tc.tile_pool(name="wstr", bufs=3))
            qsbp = est.enter_context(tc.tile_pool(name="qsbp", bufs=3))
            expwp = est.enter_context(tc.tile_pool(name="expwp", bufs=4))
            sqp = est.enter_context(tc.tile_pool(name="sqp", bufs=2))
            xsqp = est.enter_context(tc.tile_pool(name="xsqp", bufs=2))
            rzbp = est.enter_context(tc.tile_pool(name="rzbp", bufs=2))
            osbp = est.enter_context(tc.tile_pool(name="osbp", bufs=2))
            smallp = est.enter_context(tc.tile_pool(name="smallp", bufs=2))
            ctxrp = est.enter_context(tc.tile_pool(name="ctxrp", bufs=2))
            ctxtp = est.enter_context(tc.tile_pool(name="ctxtp", bufs=32))
            vtmpp = est.enter_context(tc.tile_pool(name="vtmpp", bufs=2))

            ps_mm = est.enter_context(tc.tile_pool(name="ps_mm", bufs=2, space="PSUM"))
            ps_u = est.enter_context(tc.tile_pool(name="ps_u", bufs=1, space="PSUM"))
            ps_sm = est.enter_context(tc.tile_pool(name="ps_sm", bufs=2, space="PSUM"))

            # ---- constants ----
            ident = consts.tile([128, 128], F32, tag="ident")
            nc.sync.dma_start(out=ident, in_=id_d[:, :])
            sel4 = consts.tile([128, 4], BF, tag="sel4")
            nc.sync.dma_start(out=sel4, in_=sel4_d[:, :])
            sel2 = consts.tile([128, 2], BF, tag="sel2")
            nc.sync.dma_start(out=sel2, in_=sel2_d[:, :])
            bc4 = consts.tile([4, 128], BF, tag="bc4")
            nc.sync.dma_start(out=bc4, in_=bc4_d[:, :])
            bc2 = consts.tile([2, 128], BF, tag="bc2")
            nc.sync.dma_start(out=bc2, in_=bc2_d[:, :])

            def load_chunked(d, n, tag):
                t = consts.tile([128, n], F32, tag=tag)
                nc.sync.dma_start(out=t, in_=d[:].rearrange("(o p) -> p o", p=128))
                return t

            gx_sb = load_chunked(gx_d, NXC, "gx")
            bx_sb = load_chunked(bx_d, NXC, "bx")
            gc_sb = load_chunked(gc_d, NCC, "gc")
            bc_sb = load_chunked(bc_d, NCC, "bc")
            bqs_sb = load_chunked(bq_d, NXC, "bqs")
            bkvs_sb = load_chunked(bkv_d, NKV, "bkvs")
            bps_sb = load_chunked(bp_d, NXC, "bps")

            eps_sb = consts.tile([4, 1], F32, tag="eps")
            nc.vector.memset(eps_sb, EPS)

            madd_sb = []
            for b in range(B_PER):
                m = consts.tile([128, NSC], F32, tag=f"madd{b}")
                nc.sync.dma_start(out=m, in_=madd_d[b].rearrange("(sc p) -> p sc", p=128))
                madd_sb.append(m)

            # ================= ctx transpose + groupnorm (both batches) ============
            ctxT = {}   # (b, ci) -> [128, SK] bf16 tile, normalized in place
            for b in range(B_PER):
                for ci in range(NCC):
                    ctxT[(b, ci)] = ctxtp.tile([128, SK], BF, tag="ctxT",
                                               name=f"ctxT_{b}_{ci}")
                for half in range(2):        # ctx column halves to bound raw tile size
                    for sc in range(NSC):
                        r = ctxrp.tile([128, CTXD // 2], F32, tag="ctxraw")
                        nc.sync.dma_start(
                            out=r,
                            in_=ctx_d[b, 128 * sc:128 * (sc + 1),
                                      half * (CTXD // 2):(half + 1) * (CTXD // 2)])
                        for cl in range(NCC // 2):
                            ci = half * (NCC // 2) + cl
                            pt = ps_sm.tile([128, 128], F32, tag="ps_sm")
                            nc.tensor.transpose(pt, r[:, 128 * cl:128 * (cl + 1)], ident)
                            nc.vector.tensor_copy(
                                out=ctxT[(b, ci)][:, 128 * sc:128 * (sc + 1)], in_=pt)

                # stats: groups of 64 channels x 256 -> 2 groups per 128-chunk
                stats_c = smallp.tile([2, NCC, 2], F32, tag="stats_c")
                for ci in range(NCC):
                    sq = sqp.tile([128, SK], BF, tag="csq")
                    nc.scalar.activation(out=sq, in_=ctxT[(b, ci)], func=AF.Square)
                    ps = ps_sm.tile([2, 2 * SK], F32, tag="ps_sm")
                    nc.tensor.matmul(ps[:, 0:SK], sel2, ctxT[(b, ci)][:, :],
                                     start=True, stop=True)
                    nc.tensor.matmul(ps[:, SK:2 * SK], sel2, sq[:, :],
                                     start=True, stop=True)
                    nc.vector.reduce_sum(out=stats_c[:, ci, :],
                                         in_=ps[:, :].rearrange("p (q n) -> p q n", q=2),
                                         axis=AX.X)
                # finalize -> per-channel A=rstd*gamma, B=beta-mean*A
                nelem = float(64 * SK)
                nc.vector.tensor_scalar_mul(out=stats_c, in0=stats_c, scalar1=1.0 / nelem)
                msq = smallp.tile([2, NCC], F32, tag="msq_c")
                nc.scalar.activation(out=msq, in_=stats_c[:, :, 0], func=AF.Square)
                var = smallp.tile([2, NCC], F32, tag="var_c")
                nc.vector.tensor_sub(out=var, in0=stats_c[:, :, 1], in1=msq)
                sd = smallp.tile([2, NCC], F32, tag="sd_c")
                nc.scalar.activation(out=sd, in_=var, func=AF.Sqrt, bias=eps_sb[0:2, :])
                rm = smallp.tile([2, 2, NCC], BF, tag="rm_c")
                with nc.allow_low_precision(reason="rstd O(1), bf16 matmul input"):
                    nc.vector.reciprocal(out=rm[:, 0, :], in_=sd)
                nc.vector.tensor_copy(out=rm[:, 1, :], in_=stats_c[:, :, 0])

                psab = ps_sm.tile([128, 2 * NCC], F32, tag="ps_sm")
                nc.tensor.matmul(psab[:, 0:NCC], bc2, rm[:, 0, :], start=True, stop=True)
                nc.tensor.matmul(psab[:, NCC:2 * NCC], bc2, rm[:, 1, :],
                                 start=True, stop=True)
                A_c = smallp.tile([128, NCC], F32, tag="A_c")
                nc.vector.tensor_mul(out=A_c, in0=psab[:, 0:NCC], in1=gc_sb)
                tmp_c = smallp.tile([128, NCC], F32, tag="tmp_c")
                nc.vector.tensor_mul(out=tmp_c, in0=psab[:, NCC:2 * NCC], in1=A_c)
                B_c = smallp.tile([128, NCC], F32, tag="B_c")
                nc.vector.tensor_sub(out=B_c, in0=bc_sb, in1=tmp_c)
                for ci in range(NCC):
                    nc.vector.tensor_scalar(out=ctxT[(b, ci)], in0=ctxT[(b, ci)],
                                            scalar1=A_c[:, ci:ci + 1],
                                            scalar2=B_c[:, ci:ci + 1],
                                            op0=ALU.mult, op1=ALU.add)

            # ================= KV projection (both batches share weight pass) ======
            k_sb = {}    # (b, oc) -> [128, SK] bf16
            vaug = []    # per batch [128, NSC, NH, 65] bf16
            for b in range(B_PER):
                va = vaugp.tile([128, NSC, NH, 128], BF, tag="vaug")
                nc.vector.memset(va, 1.0)
                vaug.append(va)

            for oc in range(NKV):
                wv = []
                for half in range(2):
                    wvh = wstr.tile([128, NCC // 2, 128], BF, tag="wblk",
                                    name=f"wv_{oc}_{half}")
                    nc.sync.dma_start(
                        out=wvh,
                        in_=wkv_d[1024 * half:1024 * (half + 1),
                                  128 * oc:128 * (oc + 1)].rearrange("(j p) o -> p j o", p=128))
                    wv.append(wvh)
                for b in range(B_PER):
                    ps = ps_sm.tile([128, SK], F32, tag="ps_sm")
                    for j in range(NCC):
                        nc.tensor.matmul(ps, wv[j // 8][:, j % 8, :], ctxT[(b, j)][:, :],
                                         start=(j == 0), stop=(j == NCC - 1))
                    if oc < NXC:      # k chunk
                        kt = ksbp.tile([128, SK], BF, tag="ksb", name=f"k_{b}_{oc}")
                        nc.scalar.activation(out=kt, in_=ps, func=AF.Identity,
                                             bias=bkvs_sb[:, oc:oc + 1], scale=1.0)
                        k_sb[(b, oc)] = kt
                    else:             # v chunk -> transpose into vaug slots
                        vc = oc - NXC
                        vt = vtmpp.tile([128, SK], F32, tag="vtmp")
                        nc.scalar.activation(out=vt, in_=ps, func=AF.Identity,
                                             bias=bkvs_sb[:, oc:oc + 1], scale=1.0)
                        for sc in range(NSC):
                            pt = ps_sm.tile([128, 128], F32, tag="ps_sm")
                            nc.tensor.transpose(pt, vt[:, 128 * sc:128 * (sc + 1)], ident)
                            nc.vector.tensor_copy(
                                out=vaug[b][:, sc, 2 * vc, 0:64], in_=pt[:, 0:64])
                            nc.vector.tensor_copy(
                                out=vaug[b][:, sc, 2 * vc + 1, 0:64], in_=pt[:, 64:128])

            # ================= per-batch: x norm, Q+attention, out proj ============
            for b in range(B_PER):
                # ---- load x, cast to bf16, groupnorm (groups: 32ch x 1024) ----
                x_sb = []     # fp32, kept for the residual
                xb_sb = []    # bf16, normalized in place
                for j in range(NXC):
                    xt = big.tile([128, S], F32, tag="big", name=f"x_{b}_{j}")
                    nc.sync.dma_start(out=xt, in_=x_d[b, 128 * j:128 * (j + 1), :])
                    x_sb.append(xt)
                    xb = xbp.tile([128, S], BF, tag="xb", name=f"xb_{b}_{j}")
                    nc.gpsimd.tensor_copy(out=xb, in_=xt)
                    xb_sb.append(xb)
                stats_x = smallp.tile([4, NXC, 2], F32, tag="stats_x")
                for j in range(NXC):
                    ps = ps_mm.tile([128, S], F32, tag="ps_mm")
                    for h2 in range(2):
                        sl = slice(512 * h2, 512 * (h2 + 1))
                        nc.tensor.matmul(ps[0:4, 0:512], sel4, xb_sb[j][:, sl],
                                         start=(h2 == 0), stop=(h2 == 1))
                    for h2 in range(2):
                        sl = slice(512 * h2, 512 * (h2 + 1))
                        sq = xsqp.tile([128, 512], BF, tag="xsq")
                        nc.scalar.activation(out=sq, in_=xb_sb[j][:, sl], func=AF.Square)
                        nc.tensor.matmul(ps[0:4, 512:1024], sel4, sq[:, :],
                                         start=(h2 == 0), stop=(h2 == 1))
                    nc.vector.reduce_sum(out=stats_x[:, j, :],
                                         in_=ps[0:4, :].rearrange("p (q n) -> p q n", q=2),
                                         axis=AX.X)
                nelem = float(32 * S)
                nc.vector.tensor_scalar_mul(out=stats_x, in0=stats_x, scalar1=1.0 / nelem)
                msx = smallp.tile([4, NXC], F32, tag="msq_x")
                nc.scalar.activation(out=msx, in_=stats_x[:, :, 0], func=AF.Square)
                varx = smallp.tile([4, NXC], F32, tag="var_x")
                nc.vector.tensor_sub(out=varx, in0=stats_x[:, :, 1], in1=msx)
                sdx = smallp.tile([4, NXC], F32, tag="sd_x")
                nc.scalar.activation(out=sdx, in_=varx, func=AF.Sqrt, bias=eps_sb)
                rmx = smallp.tile([4, 2, NXC], BF, tag="rm_x")
                with nc.allow_low_precision(reason="rstd O(1), bf16 matmul input"):
                    nc.vector.reciprocal(out=rmx[:, 0, :], in_=sdx)
                nc.vector.tensor_copy(out=rmx[:, 1, :], in_=stats_x[:, :, 0])

                psab = ps_sm.tile([128, 2 * NXC], F32, tag="ps_sm")
                nc.tensor.matmul(psab[:, 0:NXC], bc4, rmx[:, 0, :], start=True, stop=True)
                nc.tensor.matmul(psab[:, NXC:2 * NXC], bc4, rmx[:, 1, :],
                                 start=True, stop=True)
                A_x = smallp.tile([128, NXC], F32, tag="A_x")
                nc.vector.tensor_mul(out=A_x, in0=psab[:, 0:NXC], in1=gx_sb)
                tmp_x = smallp.tile([128, NXC], F32, tag="tmp_x")
                nc.vector.tensor_mul(out=tmp_x, in0=psab[:, NXC:2 * NXC], in1=A_x)
                B_x = smallp.tile([128, NXC], F32, tag="B_x")
                nc.vector.tensor_sub(out=B_x, in0=bx_sb, in1=tmp_x)
                for j in range(NXC):
                    nc.vector.tensor_scalar(out=xb_sb[j], in0=xb_sb[j],
                                            scalar1=A_x[:, j:j + 1],
                                            scalar2=B_x[:, j:j + 1],
                                            op0=ALU.mult, op1=ALU.add)

                # ---- Q projection interleaved with attention (per channel chunk) ----
                a_sb = []
                for j in range(NXC):
                    at = asbp.tile([128, S], BF, tag="asb", name=f"a_{b}_{j}")
                    a_sb.append(at)
                for hj in range(NXC):
                    wq = wstr.tile([128, NXC, 128], BF, tag="wblk", name=f"wq_{b}_{hj}")
                    nc.sync.dma_start(
                        out=wq,
                        in_=wq_d[:, 128 * hj:128 * (hj + 1)].rearrange("(j p) o -> p j o", p=128))
                    ps = ps_mm.tile([128, S], F32, tag="ps_mm")
                    for j in range(NXC):
                        for h2 in range(2):
                            sl = slice(512 * h2, 512 * (h2 + 1))
                            nc.tensor.matmul(ps[:, sl], wq[:, j, :], xb_sb[j][:, sl],
                                             start=(j == 0), stop=(j == NXC - 1))
                    qt = qsbp.tile([128, S], BF, tag="qsb", name=f"q_{b}_{hj}")
                    nc.scalar.activation(out=qt, in_=ps, func=AF.Identity,
                                         bias=bqs_sb[:, hj:hj + 1], scale=1.0)

                    for h in (2 * hj, 2 * hj + 1):
                        off = 64 * (h % 2)
                        expw = []
                        for sc in range(NSC):
                            psw = ps_mm.tile([128, S], F32, tag="ps_mm")
                            for h2 in range(2):
                                sl = slice(512 * h2, 512 * (h2 + 1))
                                nc.tensor.matmul(
                                    psw[:, sl],
                                    k_sb[(b, hj)][off:off + 64, 128 * sc:128 * (sc + 1)],
                                    qt[off:off + 64, sl],
                                    start=True, stop=True)
                            ew = expwp.tile([128, S], BF, tag="expw")
                            nc.scalar.activation(out=ew, in_=psw, func=AF.Exp,
                                                 bias=madd_sb[b][:, sc:sc + 1], scale=1.0)
                            expw.append(ew)
                        psu = ps_u.tile([128, S], F32, tag="ps_u")
                        for sc in range(NSC):
                            for h2 in range(2):
                                sl = slice(512 * h2, 512 * (h2 + 1))
                                nc.tensor.matmul(psu[:, sl], vaug[b][:, sc, h, :],
                                                 expw[sc][:, sl],
                                                 start=(sc == 0), stop=(sc == NSC - 1))
                        rzb = rzbp.tile([64, S], F32, tag="rzb")
                        nc.vector.reciprocal(out=rzb, in_=psu[64:128, :])
                        nc.vector.tensor_mul(out=a_sb[hj][off:off + 64, :],
                                             in0=psu[0:64, :], in1=rzb)

                # ---- output projection + bias + residual ----
                for o in range(NXC):
                    wp = wstr.tile([128, NXC, 128], BF, tag="wblk", name=f"wp_{b}_{o}")
                    nc.sync.dma_start(
                        out=wp,
                        in_=wp_d[:, 128 * o:128 * (o + 1)].rearrange("(j p) o -> p j o", p=128))
                    ps = ps_mm.tile([128, S], F32, tag="ps_mm")
                    for j in range(NXC):
                        for h2 in range(2):
                            sl = slice(512 * h2, 512 * (h2 + 1))
                            nc.tensor.matmul(ps[:, sl], wp[:, j, :], a_sb[j][:, sl],
                                             start=(j == 0), stop=(j == NXC - 1))
                    ot = osbp.tile([128, S], F32, tag="osb")
                    nc.vector.scalar_tensor_tensor(out=ot, in0=ps,
                                                   scalar=bps_sb[:, o:o + 1], in1=x_sb[o],
                                                   op0=ALU.add, op1=ALU.add)
                    nc.sync.dma_start(out=out_d[b, 128 * o:128 * (o + 1), :], in_=ot)

    nc.compile()
    return nc


def _host_prep(x, context, mask, gamma_x, beta_x, gamma_c, beta_c,
               Wq, bq, Wkv, bkv, Wp, bp):
    import ml_dtypes
    f = np.float32
    bf = ml_dtypes.bfloat16
    scale = 1.0 / np.sqrt(np.sqrt(D))
    xf = np.ascontiguousarray(x.reshape(x.shape[0], C, S), dtype=f)
    ctx = np.ascontiguousarray(context, dtype=f)
    # mask==1 -> 0.0 ; mask==0 -> -1e9
    madd = np.ascontiguousarray(((mask.astype(f) - 1.0) * 1e9).astype(f))
    wqt = np.ascontiguousarray((Wq.astype(f) * scale).T.astype(bf))
    wkv_mod = np.concatenate([Wkv[:C].astype(f) * scale, Wkv[C:].astype(f)], axis=0)
    wkvt = np.ascontiguousarray(wkv_mod.T.astype(bf))
    bkv_mod = np.concatenate([bkv[:C].astype(f) * scale, bkv[C:].astype(f)], axis=0)
    wpt = np.ascontiguousarray(Wp.astype(f).T.astype(bf))

    p = np.arange(128)
    sel4 = np.zeros((128, 4), f)
    sel4[p, p // 32] = 1.0
    sel2 = np.zeros((128, 2), f)
    sel2[p, p // 64] = 1.0

    shared = {
        "wqt": wqt, "wkvt": wkvt, "wpt": wpt,
        "bqs": np.ascontiguousarray(bq.astype(f) * scale),
        "bkvs": np.ascontiguousarray(bkv_mod.astype(f)),
        "bps": np.ascontiguousarray(bp.astype(f)),
        "gx": np.ascontiguousarray(gamma_x.astype(f)),
        "bx": np.ascontiguousarray(beta_x.astype(f)),
        "gc": np.ascontiguousarray(gamma_c.astype(f)),
        "bc": np.ascontiguousarray(beta_c.astype(f)),
        "sel4": sel4.astype(bf), "sel2": sel2.astype(bf),
        "bc4": np.ascontiguousarray(sel4.T.astype(bf)),
        "bc2": np.ascontiguousarray(sel2.T.astype(bf)),
        "ident": np.eye(128, dtype=f),
    }
    in_maps = []
    for c in range(NCORES):
        sl = slice(B_PER * c, B_PER * (c + 1))
        m = dict(shared)
        m["x"] = np.ascontiguousarray(xf[sl])
        m["ctx"] = np.ascontiguousarray(ctx[sl])
        m["madd"] = np.ascontiguousarray(madd[sl])
        in_maps.append(m)
    return in_maps


def kernel(x, context, mask, gamma_x, beta_x, gamma_c, beta_c,
           Wq, bq, Wkv, bkv, Wp, bp):
    from concourse.bass_utils import run_bass_kernel_spmd

    if "nc" not in _cache:
        _cache["nc"] = _build_program()
    nc = _cache["nc"]
    in_maps = _host_prep(x, context, mask, gamma_x, beta_x, gamma_c, beta_c,
                         Wq, bq, Wkv, bkv, Wp, bp)
    res = run_bass_kernel_spmd(nc, in_maps, list(range(NCORES)))
    outs = [res.results[c]["out"] for c in range(NCORES)]
    full = np.concatenate(outs, axis=0)          # [16, C, S]
    b, c = x.shape[0], x.shape[1]
    return full.reshape(b, c, *x.shape[2:]).astype(np.float32)
